# revision 3
# baseline (speedup 1.0000x reference)
"""Trainium2 Bass kernel for 2D inverse DWT (db1/Haar, L=2, mode='zero').

Math: with filters g0_col/g1_col (applied along H) and g0_row/g1_row (along W),
the inverse transform is purely per-pixel (stride 2, kernel length 2, no
cross-pixel mixing):

  y[2i+di, 2j+dj] = g0c[di]*g0r[dj]*low[i,j] + g1c[di]*g0r[dj]*lh[i,j]
                  + g0c[di]*g1r[dj]*hl[i,j] + g1c[di]*g1r[dj]*hh[i,j]

i.e. a 2x2 butterfly (4-point Hadamard-like transform) per pixel plus a 2x2
spatial interleave.  Sharding: data-parallel over the 256 (n,c) planes,
32 planes per NeuronCore, no cross-core communication.
"""

import sys

if "/opt/trn_rl_repo" not in sys.path:
    sys.path.insert(0, "/opt/trn_rl_repo")

import numpy as np

import concourse.bass as bass  # noqa: F401  (engine types referenced via nc)
import concourse.mybir as mybir
import concourse.tile as tile
from concourse import bacc
from concourse.bass_utils import run_bass_kernel_spmd

N_CORES = 8
N, C, H, W = 4, 64, 256, 256
PLANES = N * C                      # 256 (n,c) planes
PPC = PLANES // N_CORES             # 32 planes per core
GROUP = 2                           # planes processed per loop iteration
N_GROUPS = PPC // GROUP             # 16
PLANE = H * W                       # 65536 elems
F_IN = GROUP * PLANE // 128         # 1024 free elems per input tile
F_OUT = 2 * F_IN                    # 2048 free elems per output tile

_ADD = mybir.AluOpType.add
_SUB = mybir.AluOpType.subtract
_MUL = mybir.AluOpType.mult

_cache: dict = {}


def _sgn(x: float) -> float:
    return 1.0 if x > 0 else -1.0


def _build(weights: tuple) -> "bacc.Bacc":
    """Build + compile the per-core Bass program (same program on all cores)."""
    (a0, a1, b0, b1, c0, c1, d0, d1) = weights
    nc = bacc.Bacc("TRN2", target_bir_lowering=False, debug=False)
    f32 = mybir.dt.float32

    low_d = nc.dram_tensor("low", [PPC, PLANE], f32, kind="ExternalInput").ap()
    lh_d = nc.dram_tensor("lh", [PPC, PLANE], f32, kind="ExternalInput").ap()
    hl_d = nc.dram_tensor("hl", [PPC, PLANE], f32, kind="ExternalInput").ap()
    hh_d = nc.dram_tensor("hh", [PPC, PLANE], f32, kind="ExternalInput").ap()
    out_d = nc.dram_tensor("out", [PPC, 2 * H, 2 * W], f32, kind="ExternalOutput").ap()

    # fast path requires |g[0]| == |g[1]| per filter and nonzero leads
    fast = (
        abs(abs(a0) - abs(a1)) == 0.0
        and abs(abs(b0) - abs(b1)) == 0.0
        and abs(abs(c0) - abs(c1)) == 0.0
        and abs(abs(d0) - abs(d1)) == 0.0
        and a0 != 0.0 and b0 != 0.0 and c0 != 0.0 and d0 != 0.0
    )

    def in_view(x_d, g):
        return x_d[2 * g : 2 * g + 2].rearrange("p (q f) -> (p q) f", q=64)

    def out_view(g, di):
        v = out_d[2 * g : 2 * g + 2].rearrange("p (r two) c -> p r two c", two=2)
        v = v[:, :, di, :]
        return v.rearrange("p (q k) c -> (p q) k c", q=64)

    def half(t, dj):
        return t[:].rearrange("p (n two) -> p n two", two=2)[:, :, dj]

    with tile.TileContext(nc) as tc:
        with (
            tc.tile_pool(name="ins", bufs=3) as ip,
            tc.tile_pool(name="tmp", bufs=2) as tp,
            tc.tile_pool(name="outs", bufs=2) as op,
        ):
            for g in range(N_GROUPS):
                low_t = ip.tile([128, F_IN], f32, tag="low")
                nc.sync.dma_start(low_t[:], in_view(low_d, g))
                lh_t = ip.tile([128, F_IN], f32, tag="lh")
                nc.sync.dma_start(lh_t[:], in_view(lh_d, g))
                hl_t = ip.tile([128, F_IN], f32, tag="hl")
                nc.sync.dma_start(hl_t[:], in_view(hl_d, g))
                hh_t = ip.tile([128, F_IN], f32, tag="hh")
                nc.sync.dma_start(hh_t[:], in_view(hh_d, g))

                out0 = op.tile([128, F_OUT], f32, tag="out0")
                out1 = op.tile([128, F_OUT], f32, tag="out1")

                if fast:
                    # P(di) = sc^di*k_low*low + tc^di*k_lh*lh, etc.
                    k_low, k_lh = a0 * c0, b0 * c0
                    k_hl, k_hh = a0 * d0, b0 * d0
                    sc, tcs = _sgn(a1 / a0), _sgn(b1 / b0)
                    sr, tr = _sgn(c1 / c0), _sgn(d1 / d0)
                    op1 = _ADD if tcs > 0 else _SUB

                    # prescale lh, hh in place on the scalar engine
                    nc.scalar.mul(lh_t[:], lh_t[:], k_lh)
                    nc.scalar.mul(hh_t[:], hh_t[:], k_hh)

                    s_t = tp.tile([128, F_IN], f32, tag="s")
                    d_t = tp.tile([128, F_IN], f32, tag="d")
                    q_t = tp.tile([128, F_IN], f32, tag="q")
                    u_t = tp.tile([128, F_IN], f32, tag="u")
                    nc.vector.scalar_tensor_tensor(
                        s_t[:], low_t[:], k_low, lh_t[:], op0=_MUL, op1=_ADD)
                    nc.vector.scalar_tensor_tensor(
                        d_t[:], low_t[:], sc * k_low, lh_t[:], op0=_MUL, op1=op1)
                    nc.vector.scalar_tensor_tensor(
                        q_t[:], hl_t[:], k_hl, hh_t[:], op0=_MUL, op1=_ADD)
                    nc.vector.scalar_tensor_tensor(
                        u_t[:], hl_t[:], sc * k_hl, hh_t[:], op0=_MUL, op1=op1)

                    for di, (P, Q) in enumerate(((s_t, q_t), (d_t, u_t))):
                        ot = out0 if di == 0 else out1
                        nc.vector.tensor_tensor(
                            half(ot, 0), P[:], Q[:], op=_ADD)
                        if sr > 0:
                            nc.vector.tensor_tensor(
                                half(ot, 1), P[:], Q[:], op=_ADD if tr > 0 else _SUB)
                        elif tr > 0:
                            nc.vector.tensor_tensor(half(ot, 1), Q[:], P[:], op=_SUB)
                        else:
                            nc.vector.scalar_tensor_tensor(
                                half(ot, 1), P[:], -1.0, Q[:], op0=_MUL, op1=_SUB)
                else:
                    # general path: A(di) = g0c[di]*low + g1c[di]*lh,
                    # B(di) = g0c[di]*hl + g1c[di]*hh,
                    # y(di,dj) = g0r[dj]*A(di) + g1r[dj]*B(di)
                    g0c, g1c = (a0, a1), (b0, b1)
                    g0r, g1r = (c0, c1), (d0, d1)
                    AB = {}
                    for di in range(2):
                        for name, x0, x1 in (("A", low_t, lh_t), ("B", hl_t, hh_t)):
                            t = tp.tile([128, F_IN], f32, tag=f"gt{name}{di}")
                            nc.scalar.mul(t[:], x1[:], g1c[di])
                            r = tp.tile([128, F_IN], f32, tag=f"g{name}{di}")
                            nc.vector.scalar_tensor_tensor(
                                r[:], x0[:], g0c[di], t[:], op0=_MUL, op1=_ADD)
                            AB[(name, di)] = r
                    for di in range(2):
                        ot = out0 if di == 0 else out1
                        for dj in range(2):
                            t = tp.tile([128, F_IN], f32, tag=f"go{di}{dj}")
                            nc.scalar.mul(t[:], AB[("B", di)][:], g1r[dj])
                            nc.vector.scalar_tensor_tensor(
                                half(ot, dj), AB[("A", di)][:], g0r[dj], t[:],
                                op0=_MUL, op1=_ADD)

                nc.scalar.dma_start(
                    out_view(g, 0), out0[:].rearrange("p (k c) -> p k c", k=4))
                nc.scalar.dma_start(
                    out_view(g, 1), out1[:].rearrange("p (k c) -> p k c", k=4))

    nc.compile()
    return nc


def _get_nc(weights: tuple) -> "bacc.Bacc":
    if weights not in _cache:
        _cache[weights] = _build(weights)
    return _cache[weights]


def kernel(low, highs, g0_col, g1_col, g0_row, g1_row, _trace=False):
    low = np.asarray(low, dtype=np.float32)
    highs = np.asarray(highs, dtype=np.float32)
    g0c = np.asarray(g0_col, dtype=np.float32)
    g1c = np.asarray(g1_col, dtype=np.float32)
    g0r = np.asarray(g0_row, dtype=np.float32)
    g1r = np.asarray(g1_row, dtype=np.float32)
    assert low.shape == (N, C, H, W) and highs.shape == (N, C, 3, H, W)

    weights = (
        float(g0c[0]), float(g0c[1]), float(g1c[0]), float(g1c[1]),
        float(g0r[0]), float(g0r[1]), float(g1r[0]), float(g1r[1]),
    )
    nc = _get_nc(weights)

    low_f = low.reshape(PLANES, PLANE)
    highs_f = highs.reshape(PLANES, 3, PLANE)
    lh_f = np.ascontiguousarray(highs_f[:, 0])
    hl_f = np.ascontiguousarray(highs_f[:, 1])
    hh_f = np.ascontiguousarray(highs_f[:, 2])

    in_maps = []
    for k in range(N_CORES):
        sl = slice(k * PPC, (k + 1) * PPC)
        in_maps.append({
            "low": low_f[sl], "lh": lh_f[sl], "hl": hl_f[sl], "hh": hh_f[sl],
        })

    res = run_bass_kernel_spmd(
        nc, in_maps, core_ids=list(range(N_CORES)), trace=_trace)
    y = np.concatenate([res.results[k]["out"] for k in range(N_CORES)], axis=0)
    y = y.reshape(N, C, 2 * H, 2 * W)
    if _trace:
        return y, res
    return y


# revision 24
# speedup vs baseline: 110.7176x; 110.7176x over previous
"""Trainium2 Bass kernel for 2D inverse DWT (db1/Haar, L=2, mode='zero').

Math: with filters g0_col/g1_col (applied along H) and g0_row/g1_row (along W),
the inverse transform is purely per-pixel (stride 2, kernel length 2, no
cross-pixel mixing):

  y[2i+di, 2j+dj] = g0c[di]*g0r[dj]*low[i,j] + g1c[di]*g0r[dj]*lh[i,j]
                  + g0c[di]*g1r[dj]*hl[i,j] + g1c[di]*g1r[dj]*hh[i,j]

i.e. a 2x2 butterfly (4-point Hadamard-like transform) per pixel plus a 2x2
spatial interleave.  Sharding: data-parallel over the 256 (n,c) planes,
32 planes per NeuronCore, no cross-core communication.
"""

import sys

if "/opt/trn_rl_repo" not in sys.path:
    sys.path.insert(0, "/opt/trn_rl_repo")

import numpy as np

import concourse.bass as bass  # noqa: F401  (engine types referenced via nc)
import concourse.mybir as mybir
import concourse.tile as tile
from concourse import bacc
from concourse.bass_utils import run_bass_kernel_spmd

N_CORES = 8
N, C, H, W = 4, 64, 256, 256
PLANES = N * C                      # 256 (n,c) planes
PPC = PLANES // N_CORES             # 32 planes per core
GROUP = 2                           # planes processed per loop iteration
N_GROUPS = PPC // GROUP             # 16
PLANE = H * W                       # 65536 elems
F_IN = GROUP * PLANE // 128         # 1024 free elems per input tile
F_OUT = 2 * F_IN                    # 2048 free elems per output tile

_ADD = mybir.AluOpType.add
_SUB = mybir.AluOpType.subtract
_MUL = mybir.AluOpType.mult

_cache: dict = {}


def _sgn(x: float) -> float:
    return 1.0 if x > 0 else -1.0


def _build(weights: tuple, reps: int = 1, cfg: dict | None = None) -> "bacc.Bacc":
    """Build + compile the per-core Bass program (same program on all cores).

    reps > 1 repeats the whole workload inside one NEFF execution — used only
    for differential timing (axon dispatch overhead >> kernel time).
    """
    cfg = dict(cfg or {})
    group = cfg.get("group", GROUP)
    bufs_in = cfg.get("bufs_in", 3)
    bufs_tmp = cfg.get("bufs_tmp", 2)
    bufs_out = cfg.get("bufs_out", 2)
    gp_ops = cfg.get("gp_ops", 0)   # how many stage-2 tensor_tensor ops on gpsimd
    n_groups = PPC // group
    f_in = group * PLANE // 128
    f_out = 2 * f_in
    (a0, a1, b0, b1, c0, c1, d0, d1) = weights
    nc = bacc.Bacc("TRN2", target_bir_lowering=False, debug=False)
    f32 = mybir.dt.float32

    low_d = nc.dram_tensor("low", [PPC, PLANE], f32, kind="ExternalInput").ap()
    highs_d = nc.dram_tensor(
        "highs", [PPC, 3, PLANE], f32, kind="ExternalInput").ap()
    out_d = nc.dram_tensor("out", [PPC, 2 * H, 2 * W], f32, kind="ExternalOutput").ap()

    # fast path requires |g[0]| == |g[1]| per filter and nonzero leads
    fast = (
        abs(abs(a0) - abs(a1)) == 0.0
        and abs(abs(b0) - abs(b1)) == 0.0
        and abs(abs(c0) - abs(c1)) == 0.0
        and abs(abs(d0) - abs(d1)) == 0.0
        and a0 != 0.0 and b0 != 0.0 and c0 != 0.0 and d0 != 0.0
    )

    def in_view(x_d, g):
        return x_d[group * g : group * (g + 1)].rearrange(
            "p (q f) -> (p q) f", q=128 // group)

    def highs_view(g, p):
        # one plane: [128//group, 3, f_in]
        return highs_d[group * g + p].rearrange(
            "b (q f) -> q b f", q=128 // group)

    def out_view(g, di):
        v = out_d[group * g : group * (g + 1)].rearrange(
            "p (r two) c -> p r two c", two=2)
        v = v[:, :, di, :]
        return v.rearrange("p (q k) c -> (p q) k c", q=128 // group)

    def out_view_contig(g, h):
        # out-tile h holds, per partition, 2*group consecutive output rows
        v = out_d[group * g : group * (g + 1)].rearrange(
            "p (q hh m) c -> (p q) hh (m c)",
            q=128 // group, hh=2, m=2 * group)
        return v[:, h, :]

    def half(t, dj):
        return t[:].rearrange("p (n two) -> p n two", two=2)[:, :, dj]

    dma_only = cfg.get("dma_only", False)
    compute_only = cfg.get("compute_only", False)
    contig_out = cfg.get("contig_out", False) and (fast or dma_only)

    with tile.TileContext(nc) as tc:
        with (
            tc.tile_pool(name="ins", bufs=bufs_in) as ip,
            tc.tile_pool(name="tmp", bufs=bufs_tmp) as tp,
            tc.tile_pool(name="outs", bufs=bufs_out) as op,
            tc.tile_pool(name="static", bufs=1) as sp,
        ):
            if dma_only:
                st_out = sp.tile([128, f_out], f32, tag="st_out")
                nc.gpsimd.memset(st_out[:], 0.0)
            if compute_only:
                st_ins = []
                for nm in ("slow", "slh", "shl", "shh"):
                    t = sp.tile([128, f_in], f32, tag=nm)
                    nc.gpsimd.memset(t[:], 0.5)
                    st_ins.append(t)
            for g in range(n_groups * reps):
                g = g % n_groups
                if not compute_only:
                    low_t = ip.tile([128, f_in], f32, tag="low")
                    nc.sync.dma_start(low_t[:], in_view(low_d, g))
                    hi_t = ip.tile([128, 3 * f_in], f32, tag="highs")
                    qpp = 128 // group
                    for p in range(group):
                        nc.sync.dma_start(
                            hi_t[p * qpp:(p + 1) * qpp].rearrange(
                                "p (b f) -> p b f", b=3),
                            highs_view(g, p))
                    low_a = low_t[:]
                    lh_a = hi_t[:, 0 * f_in : 1 * f_in]
                    hl_a = hi_t[:, 1 * f_in : 2 * f_in]
                    hh_a = hi_t[:, 2 * f_in : 3 * f_in]
                else:
                    low_a, lh_a, hl_a, hh_a = (t[:] for t in st_ins)

                if dma_only:
                    if contig_out:
                        nc.scalar.dma_start(out_view_contig(g, 0), st_out[:])
                        nc.scalar.dma_start(out_view_contig(g, 1), st_out[:])
                    else:
                        nc.scalar.dma_start(
                            out_view(g, 0),
                            st_out[:].rearrange("p (k c) -> p k c", k=4))
                        nc.scalar.dma_start(
                            out_view(g, 1),
                            st_out[:].rearrange("p (k c) -> p k c", k=4))
                    continue

                out0 = op.tile([128, f_out], f32, tag="out0")
                out1 = op.tile([128, f_out], f32, tag="out1")

                if fast:
                    # P(di) = sc^di*k_low*low + tc^di*k_lh*lh, etc.
                    k_low, k_lh = a0 * c0, b0 * c0
                    k_hl, k_hh = a0 * d0, b0 * d0
                    sc, tcs = _sgn(a1 / a0), _sgn(b1 / b0)
                    sr, tr = _sgn(c1 / c0), _sgn(d1 / d0)
                    op1 = _ADD if tcs > 0 else _SUB

                    # prescale lh, hh on the scalar engine
                    lh_s = tp.tile([128, f_in], f32, tag="lh2")
                    hh_s = tp.tile([128, f_in], f32, tag="hh2")
                    nc.scalar.mul(lh_s[:], lh_a, k_lh)
                    nc.scalar.mul(hh_s[:], hh_a, k_hh)
                    lh_a, hh_a = lh_s[:], hh_s[:]

                    s_t = tp.tile([128, f_in], f32, tag="s")
                    d_t = tp.tile([128, f_in], f32, tag="d")
                    q_t = tp.tile([128, f_in], f32, tag="q")
                    u_t = tp.tile([128, f_in], f32, tag="u")
                    nc.vector.scalar_tensor_tensor(
                        s_t[:], low_a, k_low, lh_a, op0=_MUL, op1=_ADD)
                    nc.vector.scalar_tensor_tensor(
                        d_t[:], low_a, sc * k_low, lh_a, op0=_MUL, op1=op1)
                    nc.vector.scalar_tensor_tensor(
                        q_t[:], hl_a, k_hl, hh_a, op0=_MUL, op1=_ADD)
                    nc.vector.scalar_tensor_tensor(
                        u_t[:], hl_a, sc * k_hl, hh_a, op0=_MUL, op1=op1)

                    def emit_st2(dst, Ps, Qs, dj, eng):
                        # dst = sr^dj * P + tr^dj * Q
                        if dj == 0:
                            eng.tensor_tensor(dst, Ps, Qs, op=_ADD)
                        elif sr > 0:
                            eng.tensor_tensor(
                                dst, Ps, Qs, op=_ADD if tr > 0 else _SUB)
                        elif tr > 0:
                            eng.tensor_tensor(dst, Qs, Ps, op=_SUB)
                        else:
                            eng.scalar_tensor_tensor(
                                dst, Ps, -1.0, Qs, op0=_MUL, op1=_SUB)

                    if contig_out:
                        halfsz = f_in // 2
                        n_st2, idx = 8, 0
                        for h, ot in ((0, out0), (1, out1)):
                            for di, (P, Q) in enumerate(
                                    ((s_t, q_t), (d_t, u_t))):
                                for dj in (0, 1):
                                    dst = ot[:].rearrange(
                                        "p (r dd cc two) -> p r dd cc two",
                                        dd=2, cc=256, two=2)[:, :, di, :, dj]
                                    Ps = P[:, h * halfsz:(h + 1) * halfsz]\
                                        .rearrange("p (r c) -> p r c", c=256)
                                    Qs = Q[:, h * halfsz:(h + 1) * halfsz]\
                                        .rearrange("p (r c) -> p r c", c=256)
                                    eng = (nc.gpsimd if idx >= n_st2 - gp_ops
                                           else nc.vector)
                                    emit_st2(dst, Ps, Qs, dj, eng)
                                    idx += 1
                    else:
                        st2 = []
                        for di, (P, Q) in enumerate(((s_t, q_t), (d_t, u_t))):
                            ot = out0 if di == 0 else out1
                            st2.append((half(ot, 0), P[:], Q[:], 0))
                            st2.append((half(ot, 1), P[:], Q[:], 1))
                        for i, (dst, Ps, Qs, dj) in enumerate(st2):
                            eng = (nc.gpsimd if i >= len(st2) - gp_ops
                                   else nc.vector)
                            emit_st2(dst, Ps, Qs, dj, eng)
                else:
                    # general path: A(di) = g0c[di]*low + g1c[di]*lh,
                    # B(di) = g0c[di]*hl + g1c[di]*hh,
                    # y(di,dj) = g0r[dj]*A(di) + g1r[dj]*B(di)
                    g0c, g1c = (a0, a1), (b0, b1)
                    g0r, g1r = (c0, c1), (d0, d1)
                    AB = {}
                    for di in range(2):
                        for name, x0, x1 in (("A", low_a, lh_a), ("B", hl_a, hh_a)):
                            t = tp.tile([128, f_in], f32, tag=f"gt{name}{di}")
                            nc.scalar.mul(t[:], x1, g1c[di])
                            r = tp.tile([128, f_in], f32, tag=f"g{name}{di}")
                            nc.vector.scalar_tensor_tensor(
                                r[:], x0, g0c[di], t[:], op0=_MUL, op1=_ADD)
                            AB[(name, di)] = r
                    for di in range(2):
                        ot = out0 if di == 0 else out1
                        for dj in range(2):
                            t = tp.tile([128, f_in], f32, tag=f"go{di}{dj}")
                            nc.scalar.mul(t[:], AB[("B", di)][:], g1r[dj])
                            nc.vector.scalar_tensor_tensor(
                                half(ot, dj), AB[("A", di)][:], g0r[dj], t[:],
                                op0=_MUL, op1=_ADD)

                if not compute_only:
                    if contig_out:
                        nc.scalar.dma_start(out_view_contig(g, 0), out0[:])
                        nc.scalar.dma_start(out_view_contig(g, 1), out1[:])
                    else:
                        nc.scalar.dma_start(
                            out_view(g, 0),
                            out0[:].rearrange("p (k c) -> p k c", k=4))
                        nc.scalar.dma_start(
                            out_view(g, 1),
                            out1[:].rearrange("p (k c) -> p k c", k=4))

    nc.compile()
    return nc


def _get_nc(weights: tuple, reps: int = 1, cfg: dict | None = None) -> "bacc.Bacc":
    key = (weights, reps, tuple(sorted((cfg or {}).items())))
    if key not in _cache:
        _cache[key] = _build(weights, reps, cfg)
    return _cache[key]


def make_in_maps(low, highs):
    low_f = np.ascontiguousarray(low, dtype=np.float32).reshape(PLANES, PLANE)
    highs_f = np.ascontiguousarray(highs, dtype=np.float32).reshape(
        PLANES, 3, PLANE)
    in_maps = []
    for k in range(N_CORES):
        sl = slice(k * PPC, (k + 1) * PPC)
        in_maps.append({"low": low_f[sl], "highs": highs_f[sl]})
    return in_maps


def kernel(low, highs, g0_col, g1_col, g0_row, g1_row, _trace=False):
    low = np.asarray(low, dtype=np.float32)
    highs = np.asarray(highs, dtype=np.float32)
    g0c = np.asarray(g0_col, dtype=np.float32)
    g1c = np.asarray(g1_col, dtype=np.float32)
    g0r = np.asarray(g0_row, dtype=np.float32)
    g1r = np.asarray(g1_row, dtype=np.float32)
    assert low.shape == (N, C, H, W) and highs.shape == (N, C, 3, H, W)

    weights = (
        float(g0c[0]), float(g0c[1]), float(g1c[0]), float(g1c[1]),
        float(g0r[0]), float(g0r[1]), float(g1r[0]), float(g1r[1]),
    )
    nc = _get_nc(weights)

    in_maps = make_in_maps(low, highs)
    last_err = None
    for _attempt in range(3):
        try:
            res = run_bass_kernel_spmd(
                nc, in_maps, core_ids=list(range(N_CORES)), trace=_trace)
            break
        except Exception as e:  # transient NRT/axon failures: retry
            last_err = e
    else:
        raise last_err
    y = np.concatenate([res.results[k]["out"] for k in range(N_CORES)], axis=0)
    y = y.reshape(N, C, 2 * H, 2 * W)
    if _trace:
        return y, res
    return y
